# revision 3
# baseline (speedup 1.0000x reference)
"""Self-contained Trainium2 Bass kernel: sparse graph-transformer attention.

Computes, for inputs matching the reference problem:
  q,k,v = per-head linear projections of x
  prods[b,e,h] = <q[b,src_e,:,h], k[b,dst_e,:,h]> / sqrt(16)
  attention = softmax(prods, axis=1)   (softmax over all E edges)
  returns (attention (B,E,H) f32, v (B,N,16,8) f32)

Sharding: 8 cores; core c -> batch b=c//2, edge half c%2 (80000 edges).
One SPMD program; per-core differences live entirely in the inputs
(per-batch x, per-shard pre-permuted gather indices).  The softmax
denominator is combined across the two cores of a batch with a tiny
paired AllReduce.  exp() is computed without max subtraction: the dot
products here are bounded (|p| < ~1.5) so exp never overflows and the
softmax ratio is mathematically identical.
"""

import numpy as np

import concourse.bacc as bacc
import concourse.bass as bass
import concourse.mybir as mybir
import concourse.tile as tile
from concourse.bass_utils import run_bass_kernel_spmd
from concourse.masks import make_identity

# Problem dims (hardcoded per contract)
B, N, E = 4, 10000, 160000
F_IN, D_ATT, H, D_K = 64, 128, 8, 16
NCORES = 8
P = 128
ESH = E // 2                 # 80000 edges per core
T_E = 3200                   # edges per gather tile (must be %128==0 and divide ESH)
NBLK = T_E // P              # 25 free-dim blocks per gather tile
L = NBLK                     # edges per partition per tile
NODE_BLOCKS = (N + P - 1) // P   # 79
NODE_PAD = NODE_BLOCKS * P       # 10112
F16 = mybir.dt.float16
F32 = mybir.dt.float32
I16 = mybir.dt.int16


def build_nc(esh=ESH):
    n_t = esh // T_E
    idx_cols = esh // 16
    nc = bacc.Bacc(
        "TRN2", target_bir_lowering=False, debug=False, num_devices=NCORES
    )

    x_d = nc.dram_tensor("xb", [N, F_IN], F32, kind="ExternalInput").ap()
    w_d = {}
    b_d = {}
    for nm in ("q", "k", "v"):
        w_d[nm] = nc.dram_tensor(f"{nm}w", [D_ATT, F_IN], F32, kind="ExternalInput").ap()
        b_d[nm] = nc.dram_tensor(f"{nm}b", [1, D_ATT], F32, kind="ExternalInput").ap()
    idxq_d = nc.dram_tensor("idxq", [P, idx_cols], I16, kind="ExternalInput").ap()
    idxk_d = nc.dram_tensor("idxk", [P, idx_cols], I16, kind="ExternalInput").ap()

    att_d = nc.dram_tensor("att", [n_t, P, L * H], F32, kind="ExternalOutput").ap()
    v_d = nc.dram_tensor("v", [N, D_ATT], F32, kind="ExternalOutput").ap()

    with tile.TileContext(nc) as tc:
        _program(nc, tc, x_d, w_d, b_d, idxq_d, idxk_d, att_d, v_d, n_t)
    nc.compile()
    return nc


def _program(nc, tc, x_d, w_d, b_d, idxq_d, idxk_d, att_d, v_d, n_t):
    mult = mybir.AluOpType.mult
    add = mybir.AluOpType.add
    AX = mybir.AxisListType.X

    with (
        tc.tile_pool(name="const", bufs=1) as cp,
        tc.tile_pool(name="psum", bufs=3, space="PSUM") as pp,
        tc.tile_pool(name="psum1", bufs=2, space="PSUM") as pp1,
        tc.tile_pool(name="dram", bufs=1, space="DRAM") as dp,
    ):
        ident = cp.tile([P, P], F32)
        make_identity(nc, ident[:])

        # ---- weights: Wt[nm] = [W^T ; bias] as (65, 128) fp16 stationary
        wt = {}
        for nm in ("q", "k", "v"):
            w_sb = cp.tile([P, F_IN], F32, tag="wload")
            nc.sync.dma_start(w_sb[:], w_d[nm][:, :])
            w_ps = pp1.tile([P, P], F32, tag="tps")
            nc.tensor.transpose(w_ps[:F_IN, :P], w_sb[:, :], ident[:])
            wt_t = cp.tile([F_IN + 1, D_ATT], F16, tag=f"wt{nm}")
            nc.vector.tensor_copy(wt_t[:F_IN, :], w_ps[:F_IN, :P])
            b_sb = cp.tile([1, D_ATT], F32, tag="bload")
            nc.sync.dma_start(b_sb[:], b_d[nm][:, :])
            nc.vector.tensor_copy(wt_t[F_IN : F_IN + 1, :], b_sb[:])
            wt[nm] = wt_t

        # ---- xT = [x^T ; ones] as (65, NODE_PAD) fp16
        xT = cp.tile([F_IN + 1, NODE_PAD], F16)
        nc.vector.memset(xT[F_IN : F_IN + 1, :], 1.0)
        with tc.tile_pool(name="xload", bufs=3) as xp:
            for blk in range(NODE_BLOCKS):
                n0 = blk * P
                m = min(N, n0 + P) - n0
                x_sb = xp.tile([P, F_IN], F32, tag="x")
                nc.sync.dma_start(x_sb[:m, :], x_d[n0 : n0 + m, :])
                t_ps = pp1.tile([P, P], F32, tag="tps")
                nc.tensor.transpose(t_ps[:F_IN, :m], x_sb[:m, :], ident[:m, :m])
                nc.vector.tensor_copy(xT[:F_IN, n0 : n0 + m], t_ps[:F_IN, :m])

        # ---- projections: q,k tables (fp16, node-major) to DRAM; v to output
        qtab = dp.tile([NODE_PAD, D_ATT], F16)
        ktab = dp.tile([NODE_PAD, D_ATT], F16)
        with tc.tile_pool(name="proj", bufs=3) as prp:
            if NODE_PAD > N:  # zero the pad rows (never gathered, but keep them finite)
                zpad = prp.tile([P, D_ATT], F16, tag="zpad")
                nc.vector.memset(zpad[:], 0.0)
                nc.sync.dma_start(qtab[N:NODE_PAD, :], zpad[: NODE_PAD - N, :])
                nc.sync.dma_start(ktab[N:NODE_PAD, :], zpad[: NODE_PAD - N, :])
            for blk in range(NODE_BLOCKS):
                n0 = blk * P
                m = min(N, n0 + P) - n0
                for nm, tab in (("q", qtab), ("k", ktab)):
                    ps = pp.tile([P, D_ATT], F32, tag="projps")
                    nc.tensor.matmul(
                        ps[:m, :], lhsT=xT[:, n0 : n0 + m], rhs=wt[nm][:],
                        start=True, stop=True,
                    )
                    st = prp.tile([P, D_ATT], F16, tag=f"st{nm}")
                    if nm == "q":
                        nc.scalar.copy(st[:m, :], ps[:m, :])
                    else:
                        nc.vector.tensor_copy(st[:m, :], ps[:m, :])
                    nc.sync.dma_start(tab[n0 : n0 + m, :], st[:m, :])
                ps = pp.tile([P, D_ATT], F32, tag="projps")
                nc.tensor.matmul(
                    ps[:m, :], lhsT=xT[:, n0 : n0 + m], rhs=wt["v"][:],
                    start=True, stop=True,
                )
                stv = prp.tile([P, D_ATT], F32, tag="stv")
                # channel permute: v[n, d*8+h] = y[n, h*16+d]
                nc.vector.tensor_copy(
                    stv[:m, :].rearrange("p (d h) -> p d h", h=H),
                    ps[:m, :].rearrange("p (h d) -> p d h", d=D_K),
                )
                nc.sync.dma_start(v_d[n0 : n0 + m, :], stv[:m, :])

        # ---- gather + per-edge dots + exp
        exp_all = cp.tile([P, n_t * NBLK * H], F32)
        idx_q = cp.tile([P, idxq_d.shape[1]], I16, tag="idxq")
        idx_k = cp.tile([P, idxk_d.shape[1]], I16, tag="idxk")
        nc.sync.dma_start(idx_q[:], idxq_d[:, :])
        nc.sync.dma_start(idx_k[:], idxk_d[:, :])
        ic = T_E // 16  # idx cols per tile
        with tc.tile_pool(name="gat", bufs=3) as gp:
            for t in range(n_t):
                qg = gp.tile([P, NBLK, D_ATT], F16, tag="qg")
                kg = gp.tile([P, NBLK, D_ATT], F16, tag="kg")
                nc.gpsimd.dma_gather(
                    qg[:], qtab[:], idx_q[:, t * ic : (t + 1) * ic],
                    T_E, T_E, D_ATT, single_packet=False,
                )
                nc.gpsimd.dma_gather(
                    kg[:], ktab[:], idx_k[:, t * ic : (t + 1) * ic],
                    T_E, T_E, D_ATT, single_packet=False,
                )
                qk = gp.tile([P, NBLK, D_ATT], F16, tag="qk")
                nc.vector.tensor_tensor(out=qk[:], in0=qg[:], in1=kg[:], op=mult)
                pr = gp.tile([P, NBLK * H], F32, tag="pr")
                nc.vector.tensor_reduce(
                    out=pr[:].rearrange("p (b h) -> p b h", h=H),
                    in_=qk[:].rearrange("p b (h d) -> p b h d", d=D_K),
                    axis=AX, op=add,
                )
                nc.scalar.activation(
                    out=exp_all[:, t * NBLK * H : (t + 1) * NBLK * H],
                    in_=pr[:],
                    func=mybir.ActivationFunctionType.Exp,
                    scale=0.25,
                )

        # ---- softmax denominator: local partial sums -> paired AllReduce
        acc = cp.tile([P, H], F32)
        nc.vector.tensor_reduce(
            out=acc[:],
            in_=exp_all[:].rearrange("p (b h) -> p h b", h=H),
            axis=AX, op=add,
        )
        cc_in = dp.tile([P, H], F32)
        cc_out = dp.tile([P, H], F32)
        nc.sync.dma_start(cc_in[:], acc[:])
        nc.gpsimd.collective_compute(
            "AllReduce",
            mybir.AluOpType.add,
            ins=[cc_in.opt()],
            outs=[cc_out.opt()],
            replica_groups=[[0, 1], [2, 3], [4, 5], [6, 7]],
        )
        acc2 = cp.tile([P, H], F32)
        nc.sync.dma_start(acc2[:], cc_out[:])
        ones = cp.tile([P, 1], F32)
        nc.vector.memset(ones[:], 1.0)
        sums_ps = pp1.tile([1, H], F32, tag="sums")
        nc.tensor.matmul(sums_ps[:], lhsT=ones[:], rhs=acc2[:], start=True, stop=True)
        invs = cp.tile([1, H], F32)
        nc.vector.reciprocal(invs[:], sums_ps[:])
        invsb = cp.tile([P, H], F32)
        nc.gpsimd.partition_broadcast(invsb[:], invs[:])

        # ---- normalize + store
        with tc.tile_pool(name="outp", bufs=3) as op_:
            for t in range(n_t):
                ao = op_.tile([P, NBLK * H], F32, tag="ao")
                nc.vector.tensor_tensor(
                    out=ao[:].rearrange("p (b h) -> p b h", h=H),
                    in0=exp_all[:, t * NBLK * H : (t + 1) * NBLK * H].rearrange(
                        "p (b h) -> p b h", h=H
                    ),
                    in1=invsb[:, None, :].to_broadcast([P, NBLK, H]),
                    op=mult,
                )
                nc.sync.dma_start(att_d[t], ao[:])


# ------------------------------ host side ------------------------------

_NC_CACHE = {}


def _get_nc(esh=ESH):
    if esh not in _NC_CACHE:
        _NC_CACHE[esh] = build_nc(esh)
    return _NC_CACHE[esh]


def _mk_idx(ids, esh):
    """Node ids (esh,) -> wrapped+replicated int16 gather-index array (128, esh//16).

    Slot j of gather tile t fetches edge  e = t*T_E + (j%128)*L + j//128 ;
    the instruction reads slot j from idx[j%16, j//16] (within the tile's
    column slice), replicated across the 8 groups of 16 partitions.
    """
    n_t = esh // T_E
    ids_t = ids.reshape(n_t, T_E)
    j = np.arange(T_E)
    eidx = (j % P) * L + (j // P)
    slots = ids_t[:, eidx]                                   # (n_t, T_E)
    w = slots.reshape(n_t, T_E // 16, 16).transpose(0, 2, 1)  # (n_t, 16, T_E/16)
    w = np.concatenate(list(w), axis=1)                       # (16, esh/16)
    return np.ascontiguousarray(np.tile(w, (8, 1)).astype(np.int16))


def _make_in_maps(x, Qw, Qb, Kw, Kb, Vw, Vb, edge, esh):
    x = np.asarray(x, dtype=np.float32)
    weights = {
        "qw": np.ascontiguousarray(np.asarray(Qw, np.float32)),
        "qb": np.ascontiguousarray(np.asarray(Qb, np.float32).reshape(1, D_ATT)),
        "kw": np.ascontiguousarray(np.asarray(Kw, np.float32)),
        "kb": np.ascontiguousarray(np.asarray(Kb, np.float32).reshape(1, D_ATT)),
        "vw": np.ascontiguousarray(np.asarray(Vw, np.float32)),
        "vb": np.ascontiguousarray(np.asarray(Vb, np.float32).reshape(1, D_ATT)),
    }
    edge = np.asarray(edge)
    in_maps = []
    for c in range(NCORES):
        b, half = c // 2, c % 2
        es = edge[0, half * esh : (half + 1) * esh]
        ed = edge[1, half * esh : (half + 1) * esh]
        in_maps.append(
            dict(
                weights,
                xb=np.ascontiguousarray(x[b]),
                idxq=_mk_idx(es, esh),
                idxk=_mk_idx(ed, esh),
            )
        )
    return in_maps


def _assemble(results, esh):
    n_t = esh // T_E
    att = np.empty((B, 2 * esh, H), np.float32)
    v = np.empty((B, N, D_K, H), np.float32)
    for c in range(NCORES):
        b, half = c // 2, c % 2
        a = results[c]["att"].reshape(n_t, P, L, H).reshape(n_t * P * L, H)
        att[b, half * esh : (half + 1) * esh] = a
        if half == 0:
            v[b] = results[c]["v"].reshape(N, D_K, H)
    return att, v


def kernel(x, Qw, Qb, Kw, Kb, Vw, Vb, edge, **run_kwargs):
    nc = _get_nc(ESH)
    in_maps = _make_in_maps(x, Qw, Qb, Kw, Kb, Vw, Vb, edge, ESH)
    res = run_bass_kernel_spmd(nc, in_maps, core_ids=list(range(NCORES)), **run_kwargs)
    att, v = _assemble(res.results, ESH)
    kernel.last_result = res
    return att, v


# revision 17
# speedup vs baseline: 1.4417x; 1.4417x over previous
"""Self-contained Trainium2 Bass kernel: sparse graph-transformer attention.

  q,k,v = per-head linear projections of x  (B=4 batches)
  prods[b,e,h] = <q[b,src_e,:,h], k[b,dst_e,:,h]> / sqrt(16)
  attention = softmax(prods, axis=1)   (softmax over all E=160000 edges)
  returns (attention (B,E,H) f32, v (B,N,16,8) f32)

Sharding (v2): pure edge-parallel.  The edge list is shared by all 4
batches, so the q/k tables are stored batch-concatenated ((N, 4*128)
fp16) and ONE 1KB gather descriptor fetches a node's q (or k) rows for
all four batches at once.  Each core handles E/8 = 20000 edges (padded
to 20096 = 157*128 with masked dummies).  The SWDGE descriptor
generation on the Q7 (~8ns/row, measured) is the bottleneck; the
4-batch amortization cuts it 4x vs per-(batch, edge-half) sharding.
Softmax denominators are combined with a single 8-core AllReduce of the
(128, 32) per-core partial sums.  exp() needs no max subtraction: dot
products here are bounded (|p| < ~1.5).
"""

import numpy as np

import concourse.bacc as bacc
import concourse.bass as bass
import concourse.mybir as mybir
import concourse.tile as tile
from concourse.bass_utils import run_bass_kernel_spmd
from concourse.masks import make_identity

# Problem dims (hardcoded per contract)
B, N, E = 4, 10000, 160000
F_IN, D_ATT, H, D_K = 64, 128, 8, 16
NCORES = 8
P = 128
CH = B * D_ATT                  # 512 table columns (4 batches x 128 ch)
EREAL = E // NCORES             # 20000 real edges per core
NBLK_ALL = 157                  # ceil(20000/128)
ECORE = NBLK_ALL * P            # 20096 slots per core (96 dummies)
L_TILES = [16] * 9 + [13]       # gather-tile sizes in 128-slot blocks
NODE_BLOCKS = (N + P - 1) // P  # 79
NODE_PAD = NODE_BLOCKS * P      # 10112
IDX_COLS = ECORE // 16          # 1256
NCOL = B * NBLK_ALL * H         # 5024 exp columns: (b, gblk, h)
F16 = mybir.dt.float16
F32 = mybir.dt.float32
I16 = mybir.dt.int16

QG_AHEAD = 3                    # q-gathers run this many tiles ahead of k


def build_nc():
    nc = bacc.Bacc(
        "TRN2", target_bir_lowering=False, debug=False, num_devices=NCORES
    )
    x_d = nc.dram_tensor("x", [B, N, F_IN], F32, kind="ExternalInput").ap()
    w_d, b_d = {}, {}
    for nm in ("q", "k", "v"):
        w_d[nm] = nc.dram_tensor(f"{nm}w", [D_ATT, F_IN], F32, kind="ExternalInput").ap()
        b_d[nm] = nc.dram_tensor(f"{nm}b", [1, D_ATT], F32, kind="ExternalInput").ap()
    idxq_d = nc.dram_tensor("idxq", [P, IDX_COLS], I16, kind="ExternalInput").ap()
    idxk_d = nc.dram_tensor("idxk", [P, IDX_COLS], I16, kind="ExternalInput").ap()
    att_d = nc.dram_tensor("att", [B, ECORE, H], F32, kind="ExternalOutput").ap()
    v_d = nc.dram_tensor("v", [B, N, D_ATT], F32, kind="ExternalOutput").ap()

    with tile.TileContext(nc) as tc:
        _program(nc, tc, x_d, w_d, b_d, idxq_d, idxk_d, att_d, v_d)
    nc.compile()
    return nc


def _program(nc, tc, x_d, w_d, b_d, idxq_d, idxk_d, att_d, v_d):
    mult = mybir.AluOpType.mult
    add = mybir.AluOpType.add
    AX = mybir.AxisListType.X

    with (
        tc.tile_pool(name="const", bufs=1) as cp,
        tc.tile_pool(name="psum", bufs=2, space="PSUM") as pp,
        tc.tile_pool(name="psum1", bufs=2, space="PSUM") as pp1,
        tc.tile_pool(name="dram", bufs=1, space="DRAM") as dp,
        tc.tile_pool(name="xt", bufs=2) as xtp,
        tc.tile_pool(name="stage", bufs=2) as sp,
        tc.tile_pool(name="gat_q", bufs=QG_AHEAD + 1) as gq,
        tc.tile_pool(name="gat_k", bufs=2) as gk,
    )  :
        ident = cp.tile([P, P], F32)
        make_identity(nc, ident[:])

        # ---- weights: Wt[nm] = [W^T ; bias] as (65, 128) fp16
        wt = {}
        for nm in ("q", "k", "v"):
            w_sb = cp.tile([P, F_IN], F32, tag="wload")
            nc.sync.dma_start(w_sb[:], w_d[nm][:, :])
            w_ps = pp1.tile([P, P], F32, tag="tps")
            nc.tensor.transpose(w_ps[:F_IN, :P], w_sb[:, :], ident[:])
            wt_t = cp.tile([F_IN + 1, D_ATT], F16, tag=f"wt{nm}")
            nc.vector.tensor_copy(wt_t[:F_IN, :], w_ps[:F_IN, :P])
            b_sb = cp.tile([1, D_ATT], F32, tag="bload")
            nc.sync.dma_start(b_sb[:], b_d[nm][:, :])
            nc.vector.tensor_copy(wt_t[F_IN : F_IN + 1, :], b_sb[:])
            wt[nm] = wt_t

        # ---- x cast to fp16 in DRAM, padded to (NODE_PAD, 128) per batch
        zt = cp.tile([P, 16 * F_IN], F16, tag="zeros")
        nc.vector.memset(zt[:], 0.0)
        xcast = [
            dp.tile([NODE_PAD, P], F16, name=f"xcast{b}", tag=f"xcast{b}")
            for b in range(B)
        ]
        XCHUNK = 16
        for b in range(B):
            for g0 in range(0, NODE_BLOCKS, XCHUNK):
                g1 = min(g0 + XCHUNK, NODE_BLOCKS)
                n0, n1 = g0 * P, min(g1 * P, N)
                nrows = n1 - n0
                nfull = nrows // P  # full 128-blocks in this chunk
                x_sb = sp.tile([P, XCHUNK, F_IN], F32, tag="xload")
                st = sp.tile([P, XCHUNK, F_IN], F16, tag="xcast")
                if nfull:
                    nc.sync.dma_start(
                        x_sb[:, :nfull, :],
                        x_d[b, n0 : n0 + nfull * P, :].rearrange(
                            "(g p) f -> p g f", p=P
                        ),
                    )
                    nc.scalar.copy(st[:, :nfull, :], x_sb[:, :nfull, :])
                    nc.scalar.dma_start(
                        xcast[b][n0 : n0 + nfull * P, :F_IN].rearrange(
                            "(g p) f -> p g f", p=P
                        ),
                        st[:, :nfull, :],
                    )
                    # zero the col pad 64:128
                    nc.scalar.dma_start(
                        xcast[b][n0 : n0 + nfull * P, F_IN:].rearrange(
                            "(g p) f -> p g f", p=P
                        ),
                        zt[:, : nfull * F_IN].rearrange("p (g f) -> p g f", f=F_IN),
                    )
                m = nrows - nfull * P  # ragged tail rows (only last chunk: 16)
                if m:
                    r0 = n0 + nfull * P
                    x_sb2 = sp.tile([P, F_IN], F32, tag="xload2")
                    st2 = sp.tile([P, F_IN], F16, tag="xcast2")
                    nc.sync.dma_start(x_sb2[:m, :], x_d[b, r0:N, :])
                    nc.scalar.copy(st2[:m, :], x_sb2[:m, :])
                    nc.scalar.dma_start(xcast[b][r0:N, :F_IN], st2[:m, :])
                    nc.scalar.dma_start(
                        xcast[b][r0:N, F_IN:], zt[:m, :F_IN]
                    )
            # node pad rows [N, NODE_PAD) = 0
            nc.scalar.dma_start(
                xcast[b][N:NODE_PAD, :], zt[: NODE_PAD - N, :P]
            )

        # ---- tables (DRAM, fp16): row n = [q_b0 | q_b1 | q_b2 | q_b3]
        qtab = dp.tile([NODE_PAD, CH], F16)
        ktab = dp.tile([NODE_PAD, CH], F16)
        nc.scalar.dma_start(qtab[N:NODE_PAD, :], zt[: NODE_PAD - N, :CH])
        nc.scalar.dma_start(ktab[N:NODE_PAD, :], zt[: NODE_PAD - N, :CH])

        def load_xT(b):
            """(Re)build xT for batch b: (128, NODE_PAD) fp16; rows 0-63 =
            x^T, row 64 = ones, rows 65+ zeros."""
            xT = xtp.tile([P, NODE_PAD], F16, tag="xT")
            nc.sync.dma_start_transpose(xT[:], xcast[b][:, :])
            nc.vector.memset(xT[F_IN : F_IN + 1, :], 1.0)
            return xT

        GRP = 4  # node-blocks per psum bank / staged DMA

        def project(b, nm, tab, xT):
            """One projection for batch b into tab columns [b*128,(b+1)*128)."""
            for g0 in range(0, NODE_BLOCKS, GRP):
                g1 = min(g0 + GRP, NODE_BLOCKS)
                ng = g1 - g0
                ps = pp.tile([P, GRP, D_ATT], F32, tag="projps")
                for j in range(ng):
                    blk = g0 + j
                    n0 = blk * P
                    m = min(N, n0 + P) - n0
                    nc.tensor.matmul(
                        ps[:m, j, :],
                        lhsT=xT[: F_IN + 1, n0 : n0 + m],
                        rhs=wt[nm][:],
                        start=True, stop=True,
                    )
                n0, n1 = g0 * P, min(g1 * P, N)
                nfull = (n1 - n0) // P
                mtail = n1 - n0 - nfull * P
                if nm == "v":
                    st = sp.tile([P, GRP, D_ATT], F32, tag="stv")
                    if nfull:
                        nc.vector.tensor_copy(
                            st[:, :nfull, :].rearrange("p g (d h) -> p g d h", h=H),
                            ps[:, :nfull, :].rearrange("p g (h d) -> p g d h", d=D_K),
                        )
                    if mtail:
                        nc.vector.tensor_copy(
                            st[:mtail, nfull, :].rearrange("p (d h) -> p d h", h=H),
                            ps[:mtail, nfull, :].rearrange("p (h d) -> p d h", d=D_K),
                        )
                    if nfull:
                        nc.scalar.dma_start(
                            v_d[b, n0 : n0 + nfull * P, :].rearrange(
                                "(g p) c -> p g c", p=P
                            ),
                            st[:, :nfull, :],
                        )
                    if mtail:  # ragged tail (16 rows)
                        nc.scalar.dma_start(
                            v_d[b, n0 + nfull * P : n1, :],
                            st[:mtail, nfull, :],
                        )
                else:
                    st = sp.tile([P, GRP, D_ATT], F16, tag=f"st{nm}")
                    cp_op = nc.vector.tensor_copy if nm == "k" else nc.scalar.copy
                    if nfull:
                        cp_op(st[:, :nfull, :], ps[:, :nfull, :])
                    if mtail:
                        cp_op(st[:mtail, nfull, :], ps[:mtail, nfull, :])
                    if nfull:
                        nc.scalar.dma_start(
                            tab[n0 : n0 + nfull * P, b * D_ATT : (b + 1) * D_ATT]
                            .rearrange("(g p) c -> p g c", p=P),
                            st[:, :nfull, :],
                        )
                    if mtail:
                        nc.scalar.dma_start(
                            tab[n0 + nfull * P : n1, b * D_ATT : (b + 1) * D_ATT],
                            st[:mtail, nfull, :],
                        )

        # q projections first (prefix); k overlaps early q-gathers; v at end
        for b in range(B):
            project(b, "q", qtab, load_xT(b))

        # ---- gather indices
        idx_q = cp.tile([P, IDX_COLS], I16, tag="idxq")
        idx_k = cp.tile([P, IDX_COLS], I16, tag="idxk")
        nc.sync.dma_start(idx_q[:], idxq_d[:, :])
        nc.sync.dma_start(idx_k[:], idxk_d[:, :])

        exp_all = cp.tile([P, NCOL], F32)   # cols = (b, gblk, h)
        acc = cp.tile([P, B * H], F32)
        nc.vector.memset(acc[:], 0.0)

        n_t = len(L_TILES)
        bases = [0]
        for Lt in L_TILES:
            bases.append(bases[-1] + Lt)  # block bases

        qg_tiles = {}

        def q_gather(t):
            Lt = L_TILES[t]
            qg = gq.tile([P, max(L_TILES), CH], F16, tag="qg")
            nc.gpsimd.dma_gather(
                qg[:, :Lt, :], qtab[:],
                idx_q[:, bases[t] * 8 : bases[t + 1] * 8],
                Lt * P, Lt * P, CH, single_packet=False,
            )
            qg_tiles[t] = qg

        def k_gather_and_compute(t):
            Lt = L_TILES[t]
            g0 = bases[t]
            kg = gk.tile([P, max(L_TILES), CH], F16, tag="kg")
            nc.gpsimd.dma_gather(
                kg[:, :Lt, :], ktab[:],
                idx_k[:, g0 * 8 : bases[t + 1] * 8],
                Lt * P, Lt * P, CH, single_packet=False,
            )
            qg = qg_tiles.pop(t)
            # product (into kg), per-(b,h) reduce, exp
            nc.vector.tensor_tensor(
                out=kg[:, :Lt, :], in0=qg[:, :Lt, :], in1=kg[:, :Lt, :], op=mult
            )
            pr = sp.tile([P, max(L_TILES) * B * H], F32, tag="pr")
            nc.vector.tensor_reduce(
                out=pr[:, : Lt * B * H].rearrange(
                    "p (l b h) -> p l b h", b=B, h=H
                ),
                in_=kg[:, :Lt, :].rearrange("p l (b h d) -> p l b h d", b=B, d=D_K),
                axis=AX, op=add,
            )
            exp_out = (
                exp_all[:]
                .rearrange("p (b g h) -> p b g h", b=B, h=H)[:, :, g0 : g0 + Lt, :]
            )
            nc.scalar.activation(
                out=exp_out,
                in_=pr[:, : Lt * B * H].rearrange(
                    "p (l b h) -> p b l h", b=B, h=H
                ),
                func=mybir.ActivationFunctionType.Exp,
                scale=0.25,
            )
            # accumulate per-(b,h) partial sums
            tmp = sp.tile([P, B * H], F32, tag="acc_tmp")
            nc.vector.tensor_reduce(
                out=tmp[:],
                in_=exp_all[:].rearrange("p (b g h) -> p b h g", b=B, h=H)[
                    :, :, :, g0 : g0 + Lt
                ],
                axis=AX, op=add,
            )
            nc.vector.tensor_add(out=acc[:], in0=acc[:], in1=tmp[:])

        emitted_q = 0
        for _ in range(min(QG_AHEAD, n_t)):
            q_gather(emitted_q)
            emitted_q += 1
        ktab_built = False
        for t in range(n_t):
            if not ktab_built:
                for b in range(B):
                    project(b, "k", ktab, load_xT(b))
                ktab_built = True
            k_gather_and_compute(t)
            if emitted_q < n_t:
                q_gather(emitted_q)
                emitted_q += 1

        # ---- v projections (fill PE/DVE while gathers stream)
        for b in range(B):
            project(b, "v", v_d, load_xT(b))

        # ---- softmax denominator across cores
        cc_in = dp.tile([P, B * H], F32)
        cc_out = dp.tile([P, B * H], F32)
        nc.sync.dma_start(cc_in[:], acc[:])
        nc.gpsimd.collective_compute(
            "AllReduce",
            mybir.AluOpType.add,
            ins=[cc_in.opt()],
            outs=[cc_out.opt()],
            replica_groups=[list(range(NCORES))],
        )
        acc2 = cp.tile([P, B * H], F32)
        nc.sync.dma_start(acc2[:], cc_out[:])
        ones = cp.tile([P, 1], F32)
        nc.vector.memset(ones[:], 1.0)
        sums_ps = pp1.tile([1, B * H], F32, tag="sums")
        nc.tensor.matmul(sums_ps[:], lhsT=ones[:], rhs=acc2[:], start=True, stop=True)
        # each core's 96 dummy edges (pointing at zeroed pad rows) contribute
        # exp(0)=1 apiece: subtract the exact constant 8*96=768.
        sums_sb = cp.tile([1, B * H], F32)
        nc.vector.tensor_scalar_add(
            sums_sb[:], sums_ps[:], -float(NCORES * (ECORE - EREAL))
        )
        invs = cp.tile([1, B * H], F32)
        nc.vector.reciprocal(invs[:], sums_sb[:])
        invsb = cp.tile([P, B * H], F32)
        nc.gpsimd.partition_broadcast(invsb[:], invs[:])

        # ---- normalize + store
        for t in range(n_t):
            Lt = L_TILES[t]
            g0 = bases[t]
            ao = sp.tile([P, B * max(L_TILES) * H], F32, tag="ao")
            ao_v = ao[:, : B * Lt * H].rearrange("p (b l h) -> p b l h", b=B, h=H)
            nc.vector.tensor_tensor(
                out=ao_v,
                in0=exp_all[:].rearrange("p (b g h) -> p b g h", b=B, h=H)[
                    :, :, g0 : g0 + Lt, :
                ],
                in1=invsb[:].rearrange("p (b h) -> p b h", b=B)[:, :, None, :]
                .to_broadcast([P, B, Lt, H]),
                op=mult,
            )
            nc.scalar.dma_start(
                att_d[:, g0 * P : g0 * P + Lt * P, :].rearrange(
                    "b (p l) h -> p b l h", l=Lt
                ),
                ao_v,
            )


# ------------------------------ host side ------------------------------

_NC_CACHE = {}


def _get_nc():
    if "nc" not in _NC_CACHE:
        _NC_CACHE["nc"] = build_nc()
    return _NC_CACHE["nc"]


def _mk_idx(ids):
    """Node ids (ECORE,) -> wrapped+replicated int16 (128, IDX_COLS).

    Slot j of gather tile t fetches local edge  e = base_e(t) + (j%128)*L_t
    + j//128 ; idx element for slot j sits at [j%16, base_col(t) + j//16],
    replicated across the 8 groups of 16 partitions.
    """
    out = np.empty((16, IDX_COLS), np.int16)
    eb = 0
    cb = 0
    for Lt in L_TILES:
        T = Lt * P
        j = np.arange(T)
        sl = ids[eb + (j % P) * Lt + (j // P)]
        out[:, cb : cb + T // 16] = sl.reshape(T // 16, 16).T
        eb += T
        cb += T // 16
    return np.ascontiguousarray(np.tile(out, (8, 1)).astype(np.int16))


def _make_in_maps(x, Qw, Qb, Kw, Kb, Vw, Vb, edge):
    common = {
        "x": np.ascontiguousarray(np.asarray(x, np.float32)),
        "qw": np.ascontiguousarray(np.asarray(Qw, np.float32)),
        "qb": np.ascontiguousarray(np.asarray(Qb, np.float32).reshape(1, D_ATT)),
        "kw": np.ascontiguousarray(np.asarray(Kw, np.float32)),
        "kb": np.ascontiguousarray(np.asarray(Kb, np.float32).reshape(1, D_ATT)),
        "vw": np.ascontiguousarray(np.asarray(Vw, np.float32)),
        "vb": np.ascontiguousarray(np.asarray(Vb, np.float32).reshape(1, D_ATT)),
    }
    edge = np.asarray(edge)
    in_maps = []
    for c in range(NCORES):
        sl = slice(c * EREAL, (c + 1) * EREAL)
        # dummies point at a zeroed pad row -> exp contribution exactly 1.0
        es = np.full(ECORE, NODE_PAD - 1, np.int64)
        ed = np.full(ECORE, NODE_PAD - 1, np.int64)
        es[:EREAL] = edge[0, sl]
        ed[:EREAL] = edge[1, sl]
        in_maps.append(dict(common, idxq=_mk_idx(es), idxk=_mk_idx(ed)))
    return in_maps


def _assemble(results):
    att = np.empty((B, E, H), np.float32)
    for c in range(NCORES):
        att[:, c * EREAL : (c + 1) * EREAL, :] = results[c]["att"][:, :EREAL, :]
    v = results[0]["v"].reshape(B, N, D_K, H)
    return att, v


def kernel(x, Qw, Qb, Kw, Kb, Vw, Vb, edge, **run_kwargs):
    nc = _get_nc()
    in_maps = _make_in_maps(x, Qw, Qb, Kw, Kb, Vw, Vb, edge)
    res = run_bass_kernel_spmd(nc, in_maps, core_ids=list(range(NCORES)), **run_kwargs)
    att, v = _assemble(res.results)
    kernel.last_result = res
    return att, v
